# revision 9
# baseline (speedup 1.0000x reference)
"""LSTM layer kernel for Trainium2, SPMD across 8 NeuronCores.

Problem: one forward LSTM layer, T=512, B=64, D=H=512 (fp32 I/O).

Strategy: data-parallel over batch (8 sequences per core, no collectives).
Per core:
  - x @ w_ih.T (+ biases) is precomputed per 64-step block with efficient
    N=512 matmuls (x is PE-transposed on chip into [d, (t,b)] layout).
  - The sequential recurrence runs one step at a time. Gates are produced
    as out[j, b] via 64 (LDWEIGHTS + MATMUL N=8) pairs: stationary =
    w_hh.T 128x128 block (bf16, FWL), moving = h_{t-1} slice [128, 8].
    Gate layout [128 part = j%128, (j//128, b)] makes the elementwise ops
    [128, 32] shaped AND makes h_t come out in exactly the layout the next
    step's moving operand needs (no per-step transposes).
  - i/f/g/o accumulate in 4 separate PSUM banks so elementwise work on
    early gates overlaps the PE matmuls of later gates.
  - Weights/h in bf16, PSUM accumulation + cell state c in fp32.
"""

import os
import numpy as np

T, B, D, H = 512, 64, 512, 512
NCORES = 8
BL = B // NCORES  # 8 sequences per core
P = 128
KC = H // P   # 4 k-chunks of h
MC = 4 * H // P  # 16 m-chunks of gate rows
DC = D // P   # 4 d-chunks of x

_compiled = {}


def _build(t_steps, block):
    import concourse.bass as bass
    import concourse.tile as tile
    from concourse import bacc, mybir
    from concourse import masks
    from contextlib import ExitStack

    FP32 = mybir.dt.float32
    BF16 = mybir.dt.bfloat16
    AF = mybir.ActivationFunctionType

    n_blocks = (t_steps + block - 1) // block
    assert t_steps % block == 0

    nc = bacc.Bacc("TRN2", target_bir_lowering=False, debug=False,
                   num_devices=NCORES)

    x_d = nc.dram_tensor("x", [t_steps, BL, D], FP32, kind="ExternalInput").ap()
    h0_d = nc.dram_tensor("h0", [BL, H], FP32, kind="ExternalInput").ap()
    c0_d = nc.dram_tensor("c0", [BL, H], FP32, kind="ExternalInput").ap()
    wih_d = nc.dram_tensor("w_ih", [4 * H, D], FP32, kind="ExternalInput").ap()
    bih_d = nc.dram_tensor("b_ih", [4 * H], FP32, kind="ExternalInput").ap()
    whh_d = nc.dram_tensor("w_hh", [4 * H, H], FP32, kind="ExternalInput").ap()
    bhh_d = nc.dram_tensor("b_hh", [4 * H], FP32, kind="ExternalInput").ap()

    hall_d = nc.dram_tensor("h_all", [t_steps, BL, H], FP32,
                            kind="ExternalOutput").ap()
    hlast_d = nc.dram_tensor("h_last", [BL, H], FP32, kind="ExternalOutput").ap()
    clast_d = nc.dram_tensor("c_last", [BL, H], FP32, kind="ExternalOutput").ap()

    with tile.TileContext(nc) as tc, ExitStack() as ctx:
        singles = ctx.enter_context(tc.tile_pool(name="singles", bufs=1))
        wload = ctx.enter_context(tc.tile_pool(name="wload", bufs=2))
        tp_ps = ctx.enter_context(
            tc.tile_pool(name="tp_ps", bufs=1, space="PSUM"))
        xg_ps = ctx.enter_context(
            tc.tile_pool(name="xg_ps", bufs=2, space="PSUM"))
        gate_ps = ctx.enter_context(
            tc.tile_pool(name="gate_ps", bufs=1, space="PSUM"))
        xload = ctx.enter_context(tc.tile_pool(name="xload", bufs=2))
        xg_pool = ctx.enter_context(tc.tile_pool(name="xg", bufs=2))
        state = ctx.enter_context(tc.tile_pool(name="state", bufs=3))
        ew = ctx.enter_context(tc.tile_pool(name="ew", bufs=3))

        ident = singles.tile([P, P], BF16)
        masks.make_identity(nc, ident[:])

        # ---- weights: load fp32, convert bf16, PE-transpose into lhsT blocks
        # whhT[:, ((k*MC+m))*P : +P] = w_hh[m*P:(m+1)*P, k*P:(k+1)*P].T
        whhT = singles.tile([P, KC * MC * P], BF16)
        wihT = singles.tile([P, DC * MC * P], BF16)
        for (w_src, w_dst, nk) in ((whh_d, whhT, KC), (wih_d, wihT, DC)):
            for m in range(MC):
                wf = wload.tile([P, H], FP32, tag="wf")
                nc.sync.dma_start(out=wf[:], in_=w_src[m * P:(m + 1) * P, :])
                wb = wload.tile([P, H], BF16, tag="wb")
                nc.vector.tensor_copy(out=wb[:], in_=wf[:])
                for k in range(nk):
                    pt = tp_ps.tile([P, P], BF16, tag="tp")
                    nc.tensor.transpose(pt[:], wb[:, k * P:(k + 1) * P], ident[:])
                    nc.scalar.copy(
                        out=w_dst[:, (k * MC + m) * P:(k * MC + m + 1) * P],
                        in_=pt[:])

        # ---- biases: bsum[p, m] = b_ih[m*128+p] + b_hh[m*128+p]
        bi_t = wload.tile([P, MC], FP32, tag="bi")
        bh_t = wload.tile([P, MC], FP32, tag="bh")
        nc.sync.dma_start(out=bi_t[:], in_=bih_d.rearrange("(m p) -> p m", p=P))
        nc.sync.dma_start(out=bh_t[:], in_=bhh_d.rearrange("(m p) -> p m", p=P))
        bsum = singles.tile([P, MC], FP32)
        nc.vector.tensor_add(bsum[:], bi_t[:], bh_t[:])

        # ---- initial state: h/c as [p, (c k, b)] with free index = 8*k + b
        hstage = wload.tile([P, KC * BL], FP32, tag="hst")
        c_prev = state.tile([P, KC * BL], FP32, tag="c")
        for k in range(KC):
            nc.sync.dma_start(
                out=hstage[:, k * BL:(k + 1) * BL],
                in_=h0_d[:, k * P:(k + 1) * P].rearrange("b p -> p b"))
            nc.sync.dma_start(
                out=c_prev[:, k * BL:(k + 1) * BL],
                in_=c0_d[:, k * P:(k + 1) * P].rearrange("b p -> p b"))
        h_prev = state.tile([P, KC * BL], BF16, tag="h")
        nc.vector.tensor_copy(out=h_prev[:], in_=hstage[:])

        # ---- xg block machinery -------------------------------------------
        # xg_blk layout: [P, (dt block, m MC, b BL)]  (bf16, bias folded in)
        def emit_xg_block(bi):
            """Returns (tile, list-of-thunks). Each thunk emits a few ops."""
            xg_blk = xg_pool.tile([P, block * MC * BL], BF16, tag="xgblk")
            thunks = []
            rt_n = (block * BL) // P  # row-tiles of 128 rows
            xT = xload.tile([P, DC * rt_n * P], BF16, tag="xT")
            for rt in range(rt_n):
                def load_rt(rt=rt):
                    t0 = bi * block + (rt * P) // BL
                    nrows_t = P // BL
                    xs = xload.tile([P, D], FP32, tag="xs")
                    nc.sync.dma_start(
                        out=xs[:],
                        in_=x_d[t0:t0 + nrows_t, :, :].rearrange(
                            "t b d -> (t b) d"))
                    xb = xload.tile([P, D], BF16, tag="xb")
                    nc.vector.tensor_copy(out=xb[:], in_=xs[:])
                    for dc in range(DC):
                        pt = tp_ps.tile([P, P], BF16, tag="tp")
                        nc.tensor.transpose(
                            pt[:], xb[:, dc * P:(dc + 1) * P], ident[:])
                        nc.scalar.copy(
                            out=xT[:, (dc * rt_n + rt) * P:
                                   (dc * rt_n + rt + 1) * P],
                            in_=pt[:])
                thunks.append(load_rt)
            nfree = rt_n * P  # total (t, b) rows in block
            for m in range(MC):
                def mm_m(m=m):
                    xps = xg_ps.tile([P, nfree], FP32, tag="xgps")
                    for dc in range(DC):
                        nc.tensor.matmul(
                            xps[:],
                            wihT[:, (dc * MC + m) * P:(dc * MC + m + 1) * P],
                            xT[:, dc * nfree:(dc + 1) * nfree],
                            start=(dc == 0), stop=(dc == DC - 1))
                    # bias + fp32->bf16, scattered into (dt, m, b) layout
                    dst = xg_blk[:].rearrange(
                        "p (dt m b) -> p dt m b", m=MC, b=BL)[:, :, m, :]
                    nc.scalar.activation(
                        out=dst, in_=xps[:].rearrange("p (dt b) -> p dt b", b=BL),
                        func=AF.Identity, bias=bsum[:, m:m + 1])
                thunks.append(mm_m)
            return xg_blk, thunks

        # gate order: g, i, f, o  (m-chunk bases: g=8, i=0, f=4, o=12)
        GATE_MS = (("g", 2 * KC), ("i", 0), ("f", 1 * KC), ("o", 3 * KC))

        xg_cur, thunks = emit_xg_block(0)
        for th in thunks:
            th()

        h_f32 = None
        for bi in range(n_blocks):
            pending = []
            if bi + 1 < n_blocks:
                xg_next, pending = emit_xg_block(bi + 1)
                pending = list(pending)
            for dt in range(block):
                t = bi * block + dt
                xgs = xg_cur[:, dt * MC * BL:(dt + 1) * MC * BL]
                ps = {}
                for gname, mbase in GATE_MS:
                    pg = gate_ps.tile([P, KC * BL], FP32, tag="ps_" + gname)
                    ps[gname] = pg
                    for mi in range(KC):
                        m = mbase + mi
                        for k in range(KC):
                            nc.tensor.matmul(
                                pg[:, mi * BL:(mi + 1) * BL],
                                whhT[:, (k * MC + m) * P:(k * MC + m + 1) * P],
                                h_prev[:, k * BL:(k + 1) * BL],
                                start=(k == 0), stop=(k == KC - 1))
                    # interleave xg-block work for the next block into the
                    # PE instruction stream between gate groups
                    if pending and gname in ("i", "o"):
                        pending.pop(0)()
                # elementwise; xg slices: i,f,g,o at m-chunk offsets 0,4,8,12
                g_sb = ew.tile([P, KC * BL], FP32, tag="g_sb")
                nc.vector.tensor_add(g_sb[:], ps["g"][:],
                                     xgs[:, 2 * KC * BL:3 * KC * BL])
                tg = ew.tile([P, KC * BL], FP32, tag="tg")
                nc.scalar.activation(tg[:], g_sb[:], AF.Tanh)
                i_sb = ew.tile([P, KC * BL], FP32, tag="i_sb")
                nc.vector.tensor_add(i_sb[:], ps["i"][:],
                                     xgs[:, 0:KC * BL])
                si = ew.tile([P, KC * BL], FP32, tag="si")
                nc.scalar.activation(si[:], i_sb[:], AF.Sigmoid)
                ig = ew.tile([P, KC * BL], FP32, tag="ig")
                nc.vector.tensor_mul(ig[:], si[:], tg[:])
                f_sb = ew.tile([P, KC * BL], FP32, tag="f_sb")
                nc.vector.tensor_add(f_sb[:], ps["f"][:],
                                     xgs[:, KC * BL:2 * KC * BL])
                sf = ew.tile([P, KC * BL], FP32, tag="sf")
                nc.scalar.activation(sf[:], f_sb[:], AF.Sigmoid)
                fc = ew.tile([P, KC * BL], FP32, tag="fc")
                nc.vector.tensor_mul(fc[:], sf[:], c_prev[:])
                c_new = state.tile([P, KC * BL], FP32, tag="c")
                nc.vector.tensor_add(c_new[:], fc[:], ig[:])
                tcl = ew.tile([P, KC * BL], FP32, tag="tc")
                nc.scalar.activation(tcl[:], c_new[:], AF.Tanh)
                o_sb = ew.tile([P, KC * BL], FP32, tag="o_sb")
                nc.vector.tensor_add(o_sb[:], ps["o"][:],
                                     xgs[:, 3 * KC * BL:4 * KC * BL])
                so = ew.tile([P, KC * BL], FP32, tag="so")
                nc.scalar.activation(so[:], o_sb[:], AF.Sigmoid)
                h_f32 = ew.tile([P, KC * BL], FP32, tag="h_f32")
                nc.vector.tensor_mul(h_f32[:], so[:], tcl[:])
                h_new = state.tile([P, KC * BL], BF16, tag="h")
                nc.vector.tensor_copy(out=h_new[:], in_=h_f32[:])
                for k in range(KC):
                    nc.sync.dma_start(
                        out=hall_d[t, :, k * P:(k + 1) * P].rearrange(
                            "b p -> p b"),
                        in_=h_f32[:, k * BL:(k + 1) * BL])
                h_prev = h_new
                c_prev = c_new
            if bi + 1 < n_blocks:
                for th in pending:  # any leftovers
                    th()
                xg_cur = xg_next

        for k in range(KC):
            nc.sync.dma_start(
                out=hlast_d[:, k * P:(k + 1) * P].rearrange("b p -> p b"),
                in_=h_f32[:, k * BL:(k + 1) * BL])
            nc.sync.dma_start(
                out=clast_d[:, k * P:(k + 1) * P].rearrange("b p -> p b"),
                in_=c_prev[:, k * BL:(k + 1) * BL])

    nc.compile()
    return nc


def _get_nc(t_steps=T, block=64):
    key = (t_steps, block)
    if key not in _compiled:
        _compiled[key] = _build(t_steps, block)
    return _compiled[key]


def _make_in_maps(inputs):
    f32 = lambda a: np.ascontiguousarray(np.asarray(a, np.float32))
    x, h0, c0 = f32(inputs["x"]), f32(inputs["h0"]), f32(inputs["c0"])
    w_ih, b_ih = f32(inputs["w_ih"]), f32(inputs["b_ih"])
    w_hh, b_hh = f32(inputs["w_hh"]), f32(inputs["b_hh"])
    in_maps = []
    for i in range(NCORES):
        sl = slice(i * BL, (i + 1) * BL)
        in_maps.append({
            "x": np.ascontiguousarray(x[:, sl, :]),
            "h0": np.ascontiguousarray(h0[sl]),
            "c0": np.ascontiguousarray(c0[sl]),
            "w_ih": w_ih, "b_ih": b_ih, "w_hh": w_hh, "b_hh": b_hh,
        })
    return in_maps


def kernel(x, h0, c0, w_ih, b_ih, w_hh, b_hh):
    from concourse.bass_utils import run_bass_kernel_spmd

    t_steps = np.asarray(x).shape[0]
    nc = _get_nc(t_steps, 64 if t_steps % 64 == 0 else t_steps)
    in_maps = _make_in_maps(dict(x=x, h0=h0, c0=c0, w_ih=w_ih, b_ih=b_ih,
                                 w_hh=w_hh, b_hh=b_hh))
    res = run_bass_kernel_spmd(nc, in_maps, core_ids=list(range(NCORES)))
    outs = res.results
    h_all = np.concatenate([outs[i]["h_all"] for i in range(NCORES)], axis=1)
    h_last = np.concatenate([outs[i]["h_last"] for i in range(NCORES)], axis=0)
    c_last = np.concatenate([outs[i]["c_last"] for i in range(NCORES)], axis=0)
    return h_all, h_last, c_last


# revision 12
# speedup vs baseline: 3.3321x; 3.3321x over previous
"""LSTM layer kernel for Trainium2, SPMD across 8 NeuronCores.

Problem: one forward LSTM layer, T=512, B=64, D=H=512 (fp32 I/O).

Strategy: data-parallel over batch (8 sequences per core, no collectives).
Per core:
  - x @ w_ih.T (+ biases) is precomputed per 64-step block with efficient
    N=512 matmuls (x is PE-transposed on chip into [d, (t,b)] layout).
  - The sequential recurrence runs one step at a time. Gates are produced
    as out[j, b] via 64 (LDWEIGHTS + MATMUL N=8) pairs: stationary =
    w_hh.T 128x128 block (bf16, FWL), moving = h_{t-1} slice [128, 8].
    Gate layout [128 part = j%128, (j//128, b)] makes the elementwise ops
    [128, 32] shaped AND makes h_t come out in exactly the layout the next
    step's moving operand needs (no per-step transposes).
  - i/f/g/o accumulate in 4 separate PSUM banks so elementwise work on
    early gates overlaps the PE matmuls of later gates.
  - Weights/h in bf16, PSUM accumulation + cell state c in fp32.
"""

import os
import numpy as np

T, B, D, H = 512, 64, 512, 512
NCORES = 8
BL = B // NCORES  # 8 sequences per core
P = 128
KC = H // P   # 4 k-chunks of h
MC = 4 * H // P  # 16 m-chunks of gate rows
DC = D // P   # 4 d-chunks of x

_compiled = {}


def _build(t_steps, block):
    import concourse.bass as bass
    import concourse.tile as tile
    from concourse import bacc, mybir
    from concourse import masks
    from contextlib import ExitStack

    FP32 = mybir.dt.float32
    BF16 = mybir.dt.bfloat16
    AF = mybir.ActivationFunctionType

    n_blocks = (t_steps + block - 1) // block
    assert t_steps % block == 0

    nc = bacc.Bacc("TRN2", target_bir_lowering=False, debug=False,
                   num_devices=NCORES)

    x_d = nc.dram_tensor("x", [t_steps, BL, D], FP32, kind="ExternalInput").ap()
    h0_d = nc.dram_tensor("h0", [BL, H], FP32, kind="ExternalInput").ap()
    c0_d = nc.dram_tensor("c0", [BL, H], FP32, kind="ExternalInput").ap()
    wih_d = nc.dram_tensor("w_ih", [4 * H, D], FP32, kind="ExternalInput").ap()
    bih_d = nc.dram_tensor("b_ih", [4 * H], FP32, kind="ExternalInput").ap()
    whh_d = nc.dram_tensor("w_hh", [4 * H, H], FP32, kind="ExternalInput").ap()
    bhh_d = nc.dram_tensor("b_hh", [4 * H], FP32, kind="ExternalInput").ap()

    hall_d = nc.dram_tensor("h_all", [t_steps, BL, H], FP32,
                            kind="ExternalOutput").ap()
    hlast_d = nc.dram_tensor("h_last", [BL, H], FP32, kind="ExternalOutput").ap()
    clast_d = nc.dram_tensor("c_last", [BL, H], FP32, kind="ExternalOutput").ap()

    with tile.TileContext(nc) as tc, ExitStack() as ctx:
        singles = ctx.enter_context(tc.tile_pool(name="singles", bufs=1))
        wload = ctx.enter_context(tc.tile_pool(name="wload", bufs=2))
        tp_ps = ctx.enter_context(
            tc.tile_pool(name="tp_ps", bufs=1, space="PSUM"))
        xg_ps = ctx.enter_context(
            tc.tile_pool(name="xg_ps", bufs=2, space="PSUM"))
        gate_ps = ctx.enter_context(
            tc.tile_pool(name="gate_ps", bufs=1, space="PSUM"))
        xload = ctx.enter_context(tc.tile_pool(name="xload", bufs=2))
        xg_pool = ctx.enter_context(tc.tile_pool(name="xg", bufs=2))
        state = ctx.enter_context(tc.tile_pool(name="state", bufs=3))
        ew = ctx.enter_context(tc.tile_pool(name="ew", bufs=3))
        hblk_pool = ctx.enter_context(tc.tile_pool(name="hblk", bufs=2))
        ost_pool = ctx.enter_context(tc.tile_pool(name="ost", bufs=2))

        ident = singles.tile([P, P], BF16)
        masks.make_identity(nc, ident[:])

        # ---- weights: load fp32, convert bf16, PE-transpose into lhsT blocks
        # whhT[:, ((k*MC+m))*P : +P] = w_hh[m*P:(m+1)*P, k*P:(k+1)*P].T
        whhT = singles.tile([P, KC * MC * P], BF16)
        wihT = singles.tile([P, DC * MC * P], BF16)
        for (w_src, w_dst, nk) in ((whh_d, whhT, KC), (wih_d, wihT, DC)):
            for m in range(MC):
                wf = wload.tile([P, H], FP32, tag="wf")
                nc.sync.dma_start(out=wf[:], in_=w_src[m * P:(m + 1) * P, :])
                wb = wload.tile([P, H], BF16, tag="wb")
                nc.vector.tensor_copy(out=wb[:], in_=wf[:])
                for k in range(nk):
                    pt = tp_ps.tile([P, P], BF16, tag="tp")
                    nc.tensor.transpose(pt[:], wb[:, k * P:(k + 1) * P], ident[:])
                    nc.scalar.copy(
                        out=w_dst[:, (k * MC + m) * P:(k * MC + m + 1) * P],
                        in_=pt[:])

        # ---- biases: bsum[p, m] = b_ih[m*128+p] + b_hh[m*128+p]
        bi_t = wload.tile([P, MC], FP32, tag="bi")
        bh_t = wload.tile([P, MC], FP32, tag="bh")
        nc.sync.dma_start(out=bi_t[:], in_=bih_d.rearrange("(m p) -> p m", p=P))
        nc.sync.dma_start(out=bh_t[:], in_=bhh_d.rearrange("(m p) -> p m", p=P))
        bsum = singles.tile([P, MC], FP32)
        nc.vector.tensor_add(bsum[:], bi_t[:], bh_t[:])

        # ---- initial state: h/c as [p, (c k, b)] with free index = 8*k + b
        hstage = wload.tile([P, KC * BL], FP32, tag="hst")
        c_prev = state.tile([P, KC * BL], FP32, tag="c")
        for k in range(KC):
            nc.sync.dma_start(
                out=hstage[:, k * BL:(k + 1) * BL],
                in_=h0_d[:, k * P:(k + 1) * P].rearrange("b p -> p b"))
            nc.sync.dma_start(
                out=c_prev[:, k * BL:(k + 1) * BL],
                in_=c0_d[:, k * P:(k + 1) * P].rearrange("b p -> p b"))
        h0sb = state.tile([P, KC * BL], BF16, tag="h")
        nc.vector.tensor_copy(out=h0sb[:], in_=hstage[:])
        h_slice = lambda k: h0sb[:, k * BL:(k + 1) * BL]
        h_full = h0sb[:]

        # ---- xg block machinery -------------------------------------------
        # xg_blk layout: [P, (dt block, m MC, b BL)]  (bf16, bias folded in)
        def emit_xg_block(bi):
            """Returns (tile, list-of-thunks). Each thunk emits a few ops."""
            xg_blk = xg_pool.tile([P, block * MC * BL], BF16, tag="xgblk")
            thunks = []
            rt_n = (block * BL) // P  # row-tiles of 128 rows
            xT = xload.tile([P, DC * rt_n * P], BF16, tag="xT")
            for rt in range(rt_n):
                def load_rt(rt=rt):
                    t0 = bi * block + (rt * P) // BL
                    nrows_t = P // BL
                    xs = xload.tile([P, D], FP32, tag="xs")
                    nc.sync.dma_start(
                        out=xs[:],
                        in_=x_d[t0:t0 + nrows_t, :, :].rearrange(
                            "t b d -> (t b) d"))
                    xb = xload.tile([P, D], BF16, tag="xb")
                    nc.vector.tensor_copy(out=xb[:], in_=xs[:])
                    for dc in range(DC):
                        pt = tp_ps.tile([P, P], BF16, tag="tp")
                        nc.tensor.transpose(
                            pt[:], xb[:, dc * P:(dc + 1) * P], ident[:])
                        nc.scalar.copy(
                            out=xT[:, (dc * rt_n + rt) * P:
                                   (dc * rt_n + rt + 1) * P],
                            in_=pt[:])
                thunks.append(load_rt)
            nfree = rt_n * P  # total (t, b) rows in block
            for m in range(MC):
                def mm_m(m=m):
                    xps = xg_ps.tile([P, nfree], FP32, tag="xgps")
                    for dc in range(DC):
                        nc.tensor.matmul(
                            xps[:],
                            wihT[:, (dc * MC + m) * P:(dc * MC + m + 1) * P],
                            xT[:, dc * nfree:(dc + 1) * nfree],
                            start=(dc == 0), stop=(dc == DC - 1))
                    # bias + fp32->bf16, scattered into (dt, m, b) layout
                    dst = xg_blk[:].rearrange(
                        "p (dt m b) -> p dt m b", m=MC, b=BL)[:, :, m, :]
                    nc.scalar.activation(
                        out=dst, in_=xps[:].rearrange("p (dt b) -> p dt b", b=BL),
                        func=AF.Identity, bias=bsum[:, m:m + 1])
                thunks.append(mm_m)
            return xg_blk, thunks

        # gate order: g, i, f, o  (m-chunk bases: g=8, i=0, f=4, o=12)
        GATE_MS = (("g", 2 * KC), ("i", 0), ("f", 1 * KC), ("o", 3 * KC))

        xg_cur, thunks = emit_xg_block(0)
        for th in thunks:
            th()

        h_f32 = None
        def flush_h_block(bi, hstage_blk):
            # staging layout [P, (k, dt, b)]: per (rb, k) transpose input is
            # the contiguous slice [:, k*block*BL + rb*P : +P]
            rt_n = (block * BL) // P
            nt = P // BL  # timesteps per row-tile
            for rb in range(rt_n):
                ost = ost_pool.tile([P, KC * P], FP32, tag="ost")
                for k in range(KC):
                    off = k * block * BL + rb * P
                    pt = tp_ps.tile([P, P], BF16, tag="otp")
                    nc.tensor.transpose(
                        pt[:], hstage_blk[:, off:off + P], ident[:])
                    nc.vector.tensor_copy(
                        out=ost[:, k * P:(k + 1) * P], in_=pt[:])
                t0 = bi * block + rb * nt
                nc.sync.dma_start(
                    out=hall_d[t0:t0 + nt, :, :].rearrange("t b f -> (t b) f"),
                    in_=ost[:])

        for bi in range(n_blocks):
            pending = []
            if bi + 1 < n_blocks:
                xg_next, pending = emit_xg_block(bi + 1)
                pending = list(pending)
            hstage_blk = hblk_pool.tile([P, block * KC * BL], BF16,
                                        tag="hstage")
            for dt in range(block):
                t = bi * block + dt
                xgs = xg_cur[:, dt * MC * BL:(dt + 1) * MC * BL]
                ps = {}
                for gname, mbase in GATE_MS:
                    pg = gate_ps.tile([P, KC * BL], FP32, tag="ps_" + gname)
                    ps[gname] = pg
                    for mi in range(KC):
                        m = mbase + mi
                        for k in range(KC):
                            nc.tensor.matmul(
                                pg[:, mi * BL:(mi + 1) * BL],
                                whhT[:, (k * MC + m) * P:(k * MC + m + 1) * P],
                                h_slice(k),
                                start=(k == 0), stop=(k == KC - 1))
                    # interleave xg-block work for the next block into the
                    # PE instruction stream between gate groups
                    if pending and gname in ("i", "o"):
                        pending.pop(0)()
                # elementwise; xg slices: i,f,g,o at m-chunk offsets 0,4,8,12
                g_sb = ew.tile([P, KC * BL], FP32, tag="g_sb")
                nc.vector.tensor_add(g_sb[:], ps["g"][:],
                                     xgs[:, 2 * KC * BL:3 * KC * BL])
                tg = ew.tile([P, KC * BL], FP32, tag="tg")
                nc.scalar.activation(tg[:], g_sb[:], AF.Tanh)
                i_sb = ew.tile([P, KC * BL], FP32, tag="i_sb")
                nc.vector.tensor_add(i_sb[:], ps["i"][:],
                                     xgs[:, 0:KC * BL])
                si = ew.tile([P, KC * BL], FP32, tag="si")
                nc.scalar.activation(si[:], i_sb[:], AF.Sigmoid)
                ig = ew.tile([P, KC * BL], FP32, tag="ig")
                nc.vector.tensor_mul(ig[:], si[:], tg[:])
                f_sb = ew.tile([P, KC * BL], FP32, tag="f_sb")
                nc.vector.tensor_add(f_sb[:], ps["f"][:],
                                     xgs[:, KC * BL:2 * KC * BL])
                sf = ew.tile([P, KC * BL], FP32, tag="sf")
                nc.scalar.activation(sf[:], f_sb[:], AF.Sigmoid)
                fc = ew.tile([P, KC * BL], FP32, tag="fc")
                nc.vector.tensor_mul(fc[:], sf[:], c_prev[:])
                c_new = state.tile([P, KC * BL], FP32, tag="c")
                nc.vector.tensor_add(c_new[:], fc[:], ig[:])
                tcl = ew.tile([P, KC * BL], FP32, tag="tc")
                nc.scalar.activation(tcl[:], c_new[:], AF.Tanh)
                o_sb = ew.tile([P, KC * BL], FP32, tag="o_sb")
                nc.vector.tensor_add(o_sb[:], ps["o"][:],
                                     xgs[:, 3 * KC * BL:4 * KC * BL])
                so = ew.tile([P, KC * BL], FP32, tag="so")
                nc.scalar.activation(so[:], o_sb[:], AF.Sigmoid)
                hsv = hstage_blk[:].rearrange(
                    "p (k dt b) -> p k dt b", k=KC, b=BL)
                nc.vector.tensor_mul(
                    hsv[:, :, dt, :],
                    so[:].rearrange("p (k b) -> p k b", b=BL),
                    tcl[:].rearrange("p (k b) -> p k b", b=BL))
                h_slice = (lambda hb, d: lambda k:
                           hb[:, (k * block + d) * BL:(k * block + d + 1) * BL]
                           )(hstage_blk, dt)
                h_full = None
                c_prev = c_new
            flush_h_block(bi, hstage_blk)
            if bi + 1 < n_blocks:
                for th in pending:  # any leftovers
                    th()
                xg_cur = xg_next

        hl32 = ew.tile([P, KC * BL], FP32, tag="hl32")
        for k in range(KC):
            nc.vector.tensor_copy(out=hl32[:, k * BL:(k + 1) * BL],
                                  in_=h_slice(k))
        for k in range(KC):
            nc.sync.dma_start(
                out=hlast_d[:, k * P:(k + 1) * P].rearrange("b p -> p b"),
                in_=hl32[:, k * BL:(k + 1) * BL])
            nc.sync.dma_start(
                out=clast_d[:, k * P:(k + 1) * P].rearrange("b p -> p b"),
                in_=c_prev[:, k * BL:(k + 1) * BL])

    nc.compile()
    return nc


def _get_nc(t_steps=T, block=64):
    key = (t_steps, block)
    if key not in _compiled:
        _compiled[key] = _build(t_steps, block)
    return _compiled[key]


def _make_in_maps(inputs):
    f32 = lambda a: np.ascontiguousarray(np.asarray(a, np.float32))
    x, h0, c0 = f32(inputs["x"]), f32(inputs["h0"]), f32(inputs["c0"])
    w_ih, b_ih = f32(inputs["w_ih"]), f32(inputs["b_ih"])
    w_hh, b_hh = f32(inputs["w_hh"]), f32(inputs["b_hh"])
    in_maps = []
    for i in range(NCORES):
        sl = slice(i * BL, (i + 1) * BL)
        in_maps.append({
            "x": np.ascontiguousarray(x[:, sl, :]),
            "h0": np.ascontiguousarray(h0[sl]),
            "c0": np.ascontiguousarray(c0[sl]),
            "w_ih": w_ih, "b_ih": b_ih, "w_hh": w_hh, "b_hh": b_hh,
        })
    return in_maps


def kernel(x, h0, c0, w_ih, b_ih, w_hh, b_hh):
    from concourse.bass_utils import run_bass_kernel_spmd

    t_steps = np.asarray(x).shape[0]
    nc = _get_nc(t_steps, 64 if t_steps % 64 == 0 else t_steps)
    in_maps = _make_in_maps(dict(x=x, h0=h0, c0=c0, w_ih=w_ih, b_ih=b_ih,
                                 w_hh=w_hh, b_hh=b_hh))
    res = run_bass_kernel_spmd(nc, in_maps, core_ids=list(range(NCORES)))
    outs = res.results
    h_all = np.concatenate([outs[i]["h_all"] for i in range(NCORES)], axis=1)
    h_last = np.concatenate([outs[i]["h_last"] for i in range(NCORES)], axis=0)
    c_last = np.concatenate([outs[i]["c_last"] for i in range(NCORES)], axis=0)
    return h_all, h_last, c_last
